# revision 7
# baseline (speedup 1.0000x reference)
"""Local (windowed) attention with shared KV head — TRN2 Bass kernel.

Problem: b=1, L=4096, d_model=1024, n_head=16, d_head=64, w=512.
  qp = (q@Wq)/8; k,v = kv@Wkv; per 512-chunk attention over {prev,self,next}
  chunks with zero-padded edges (softmax includes exp(0)=1 terms for pads);
  out = ctx @ Wo.

Sharding: sequence-parallel over the 8 chunks, one chunk per NeuronCore.
Each core recomputes the K/V projection for its 3-chunk halo (no
collectives). Edge cores receive zero-filled halo slices, which reproduces
the reference's zero-padding exactly.

The softmax exp of the 16x512x1536 score tensor per core is the ScalarE
wall (~107us on ScalarE alone), while the PE matmul stream needs ~95us.
To get both under the PE roofline the exp work is split across engines:
  - ScalarE: exact exp ACTIVATE on 7 of 12 y-tiles per head pair
  - DVE: Schraudolph bit-trick exp on the other 5 y-tiles: one
    tensor_scalar computing int16(score*S + 16248.875), S=128*log2(e),
    whose bit pattern IS bf16 exp(score) to within ~1.8% rms (rel-err
    budget holds: measured 1.2e-2 vs the 2e-2 gate).
Other changes vs the ScalarE-paced version: big prioritized input DMAs
across 4 queues (first exp at ~8us instead of 38us), z rows ride the bf16
ctx staging copy, reciprocal_approx_fast instead of 6-pass reciprocal,
out-proj in two 4-bank waves with interleaved psum copies/stores.
"""

import os
import numpy as np

B, L, DM, NH, DH, W = 1, 4096, 1024, 16, 64, 512
NCORES = 8
CH = L // NCORES        # 512 tokens per core
YW = 3 * W              # 1536 halo positions
P = 128
NF = DM // P            # 8 feature tiles
NY = YW // P            # 12 y tiles
NPAIR = NH // 2         # 8 head pairs

SCALE = float(P / np.log(2.0))          # 128*log2(e) folded into Wq
INV_SCALE = float(1.0 / SCALE)
MAGIC = 16248.875                        # Schraudolph bf16 magic
DVE_Y = tuple(int(x) for x in os.environ.get("KDVE", "1,3,6,8,10").split(",")
              if x != "")                # y-tiles exp'd on DVE per pair

_CACHE = {}


def _zrow(hh):
    # heads 12-15 live at partitions 32-35 so both reciprocal batches
    # start at a 32-aligned partition base (engine-op alignment rule)
    return hh if hh < 12 else 32 + (hh - 12)


def _build():
    import concourse.mybir as mybir
    import concourse.tile as tile
    from concourse import bacc
    from concourse.masks import make_identity
    from contextlib import ExitStack

    F32 = mybir.dt.float32
    BF16 = mybir.dt.bfloat16
    I16 = mybir.dt.int16
    EXP = mybir.ActivationFunctionType.Exp
    COPY = mybir.ActivationFunctionType.Copy

    nc = bacc.Bacc("TRN2", target_bir_lowering=False, debug=False)
    QT = nc.dram_tensor("QT", [DM, CH], BF16, kind="ExternalInput")
    ESEL = nc.dram_tensor("ESEL", [36, NH * 64], BF16, kind="ExternalInput")
    KVT = nc.dram_tensor("KVT", [DM, YW], BF16, kind="ExternalInput")
    WQ = nc.dram_tensor("WQ", [DM, DM], BF16, kind="ExternalInput")     # pre-scaled by S/8
    WVKP = nc.dram_tensor("WVKP", [P, DM], BF16, kind="ExternalInput")  # packed [Wv|Wk] tiles
    WO = nc.dram_tensor("WO", [DM, DM], BF16, kind="ExternalInput")
    OUT = nc.dram_tensor("OUT", [CH, DM], F32, kind="ExternalOutput")

    with tile.TileContext(nc) as tc, ExitStack() as ctx:
        perm = ctx.enter_context(tc.tile_pool(name="perm", bufs=1))

        identb = perm.tile([64, 64], F32, tag="identb")
        make_identity(nc, identb[:])
        esel = perm.tile([36, NH * 64], BF16, tag="esel")

        # --- persistent SBUF tiles
        wvkp = perm.tile([P, DM], BF16, tag="wvkp")
        wq = [perm.tile([P, DM], BF16, tag=f"wq{f}", name=f"wq{f}") for f in range(NF)]
        wo = [perm.tile([P, DM], BF16, tag=f"wo{f}", name=f"wo{f}") for f in range(NF)]
        k3T2 = perm.tile([P, YW], BF16, tag="k3T2")
        vTs = perm.tile([64, YW], F32, tag="vTs")
        v65 = [perm.tile([P, 65], BF16, tag=f"v65_{t}", name=f"v65_{t}") for t in range(NY)]
        qpT = [perm.tile([P, CH], BF16, tag=f"qpT{m}", name=f"qpT{m}") for m in range(NF)]
        ctxn = [perm.tile([P, CH], BF16, tag=f"ctxn{i}", name=f"ctxn{i}") for i in range(NPAIR)]
        cxs = [perm.tile([P, W], BF16, tag=f"cxs{h}", name=f"cxs{h}") for h in range(NH)]
        zr16b = perm.tile([36, W], BF16, tag="zr16b")   # z rows (bf16, via cxs)
        zr32 = perm.tile([36, W], F32, tag="zr32")
        zi32 = perm.tile([36, W], F32, tag="zi32")
        zi16b = perm.tile([36, W], BF16, tag="zi16b")

        # HAM warmup: dense dummy matmuls during the DMA fill open the PE
        # clock gate (needs ~3.4us of sustained activity)
        wtile = perm.tile([P, W], BF16, tag="wtile")
        nc.gpsimd.memset(wtile[:], 1.0)
        with tc.tile_pool(name="wmps", bufs=1, space="PSUM") as wmp:
            wps = wmp.tile([P, W], F32, tag="wm")
            for _ in range(6):
                nc.tensor.matmul(wps[:], wtile[:, 0:P], wtile[:],
                                 start=True, stop=True)

        with tc.tile_pool(name="qt", bufs=1) as qtp, \
             tc.tile_pool(name="qpps", bufs=1, space="PSUM") as qpp, \
             tc.tile_pool(name="zn", bufs=4) as znp:
            qt = [qtp.tile([P, CH], BF16, tag=f"qt{f}", name=f"qt{f}") for f in range(NF)]

            def qproj(m):
                ps = qpp.tile([P, CH], F32, tag="qp")
                for f in range(NF):
                    nc.tensor.matmul(ps[:], wq[f][:, P * m:P * (m + 1)], qt[f][:],
                                     start=(f == 0), stop=(f == NF - 1))
                with nc.allow_low_precision(reason="bf16 attention pipeline"):
                    nc.vector.tensor_copy(qpT[m][:], ps[:])

            with tc.tile_pool(name="kvt", bufs=1) as kvtp, \
                 tc.tile_pool(name="ph0ps", bufs=2, space="PSUM") as ph0, \
                 tc.tile_pool(name="tpps", bufs=2, space="PSUM") as tpp:
                kvt = [kvtp.tile([P, YW], BF16, tag=f"kvt{f}", name=f"kvt{f}")
                       for f in range(NF)]

                # --- input DMA schedule, 3 queues, critical-first
                # sync: wvkp, then kvt chunk-ordered low feature half
                nc.sync.dma_start(wvkp[:], WVKP.ap()[:, :])
                for n in range(3):
                    ns_ = slice(W * n, W * (n + 1))
                    for f in range(NF // 2):
                        nc.sync.dma_start(kvt[f][:, ns_],
                                          KVT.ap()[P * f:P * (f + 1), ns_])
                # gpsimd: kvt high half chunk-ordered interleaved with qt low
                ns_ = slice(0, W)
                for f in range(NF // 2, NF):
                    nc.gpsimd.dma_start(kvt[f][:, ns_],
                                        KVT.ap()[P * f:P * (f + 1), ns_])
                for f in range(NF // 2):
                    nc.gpsimd.dma_start(qt[f][:], QT.ap()[P * f:P * (f + 1), :])
                for n in (1, 2):
                    ns_ = slice(W * n, W * (n + 1))
                    for f in range(NF // 2, NF):
                        nc.gpsimd.dma_start(kvt[f][:, ns_],
                                            KVT.ap()[P * f:P * (f + 1), ns_])
                nc.gpsimd.dma_start(esel[:], ESEL.ap()[:, :])
                for f in range(NF):
                    nc.gpsimd.dma_start(wo[f][:], WO.ap()[P * f:P * (f + 1), :])
                # scalar: wq full rows (qproj(0) needs all f), then qt high
                for f in range(NF):
                    nc.scalar.dma_start(wq[f][:], WQ.ap()[P * f:P * (f + 1), :])
                for f in range(NF // 2, NF):
                    nc.scalar.dma_start(qt[f][:], QT.ap()[P * f:P * (f + 1), :])

                def kvproj(n):
                    ps = ph0.tile([P, W], F32, tag="kvp")
                    for f in range(NF):
                        nc.tensor.matmul(ps[:], wvkp[:, P * f:P * (f + 1)],
                                         kvt[f][:, W * n:W * (n + 1)],
                                         start=(f == 0), stop=(f == NF - 1))
                    ns = slice(W * n, W * (n + 1))
                    with nc.allow_low_precision(reason="bf16 attention pipeline"):
                        nc.vector.tensor_copy(vTs[:, ns], ps[0:64, :])
                        nc.vector.tensor_copy(k3T2[64:128, ns], ps[64:128, :])
                    # duplicate kT into the low partition half (partition remap)
                    nc.scalar.dma_start(k3T2[0:64, ns], k3T2[64:128, ns])
                    # v65 tiles for this chunk: PE transpose of vT + ones col
                    for t in range(4 * n, 4 * (n + 1)):
                        tp = tpp.tile([P, 64], F32, tag="tp")
                        nc.tensor.transpose(tp[:], vTs[:, P * t:P * (t + 1)],
                                            identb[:])
                        with nc.allow_low_precision(reason="bf16 attention pipeline"):
                            nc.vector.tensor_copy(v65[t][:, 0:64], tp[:])
                        nc.vector.memset(v65[t][:, 64:65], 1.0)

                # interleave kv-proj w-chunks with the first q-proj tiles
                kvproj(0)
                qproj(0)
                kvproj(1)
                qproj(1)
                kvproj(2)

            def z_recip(lo, hi):
                # zr16b rows -> fp32 -> 1/x -> bf16 (for the esel broadcast)
                nc.vector.tensor_copy(zr32[lo:hi, :], zr16b[lo:hi, :])
                nc.vector.reciprocal(zi32[lo:hi, :], zr32[lo:hi, :])
                with nc.allow_low_precision(reason="softmax denom"):
                    nc.vector.tensor_copy(zi16b[lo:hi, :], zi32[lo:hi, :])

            def z_apply(heads):
                for hh in heads:
                    i, h = hh // 2, hh % 2
                    lo = 0 if hh < 12 else 32
                    hi = 12 if hh < 12 else 36
                    zb = qpp.tile([P, W], F32, tag="qp")
                    nc.tensor.matmul(zb[0:64, :],
                                     esel[lo:hi, 64 * hh:64 * (hh + 1)],
                                     zi16b[lo:hi, :], start=True, stop=True)
                    if h == 0:
                        with nc.allow_low_precision(reason="bf16 ctx"):
                            nc.vector.tensor_mul(ctxn[i][0:64, :],
                                                 cxs[hh][0:64, :], zb[0:64, :])
                    else:
                        cbt = znp.tile([64, W], BF16, tag="cbt")
                        with nc.allow_low_precision(reason="bf16 ctx"):
                            nc.vector.tensor_mul(cbt[:], cxs[hh][0:64, :],
                                                 zb[0:64, :])
                        nc.sync.dma_start(ctxn[i][64:128, :], cbt[:])

            # --- attention per head pair; scores for the two heads interleave
            # into one psum tile; exp split across ScalarE (exact) and DVE
            # (Schraudolph int16 bit trick)
            attn = ExitStack()
            scp = attn.enter_context(tc.tile_pool(name="scps", bufs=2, space="PSUM"))
            cxp = attn.enter_context(tc.tile_pool(name="cxps", bufs=3, space="PSUM"))
            ptp = attn.enter_context(tc.tile_pool(name="pt", bufs=4))
            for i in range(NPAIR):
                cxA = cxp.tile([P, W], F32, tag="cx")
                cxB = cxp.tile([P, W], F32, tag="cx")
                for y in range(NY):
                    ys = slice(P * y, P * (y + 1))
                    sc = scp.tile([P, 2 * W], F32, tag="sc")
                    nc.tensor.matmul(sc[:, 0:W], k3T2[0:64, ys],
                                     qpT[i][0:64, :], start=True, stop=True,
                                     tile_position=(0, 0))
                    nc.tensor.matmul(sc[:, W:2 * W], k3T2[64:128, ys],
                                     qpT[i][64:128, :], start=True, stop=True,
                                     tile_position=(64, 0))
                    pab = ptp.tile([P, 2 * W], BF16, tag="pt")
                    with nc.allow_low_precision(reason="bf16 probs"):
                        if y in DVE_Y:
                            nc.vector.tensor_scalar(
                                pab[:].bitcast(I16), sc[:], SCALE, MAGIC,
                                op0=mybir.AluOpType.mult,
                                op1=mybir.AluOpType.add)
                        else:
                            nc.scalar.activation(pab[:], sc[:], EXP)
                    st = (y == 0)
                    sp = (y == NY - 1)
                    nc.tensor.matmul(cxA[0:65, :], v65[y][:], pab[:, 0:W],
                                     start=st, stop=sp)
                    nc.tensor.matmul(cxB[0:65, :], v65[y][:], pab[:, W:2 * W],
                                     start=st, stop=sp)
                # stage unnormalized ctx + Z row out of PSUM (bf16, one copy
                # per head; row 64 is the Z row from the v65 ones column)
                for h, cx in ((0, cxA), (1, cxB)):
                    hh = 2 * i + h
                    with nc.allow_low_precision(reason="bf16 ctx"):
                        if h == 0:
                            nc.scalar.activation(cxs[hh][0:65, :], cx[0:65, :],
                                                 COPY)
                        else:
                            nc.vector.tensor_copy(cxs[hh][0:65, :], cx[0:65, :])
                    nc.sync.dma_start(zr16b[_zrow(hh):_zrow(hh) + 1, :],
                                      cxs[hh][64:65, :])
                if i + 2 < NF:
                    qproj(i + 2)
                if i == 5:
                    z_recip(0, 12)          # heads 0-11, overlaps pair 6
                if i == 6:
                    z_apply(list(range(8)))  # overlaps pair 7
            attn.close()
            z_apply([8, 9, 10, 11])
            z_recip(32, 36)                  # heads 12-15
            z_apply([12, 13, 14, 15])

            # --- output projection, two 4-bank waves; i-outer within a wave
            with tc.tile_pool(name="opps", bufs=4, space="PSUM") as opp, \
                 tc.tile_pool(name="osb", bufs=8) as osb:
                allblk = [(x, o) for x in range(4) for o in range(2)]
                for wv in range(2):
                    blocks = allblk[4 * wv:4 * (wv + 1)]
                    pso = [opp.tile([P, W], F32, tag="op", name=f"op{wv}_{b}")
                           for b in range(4)]
                    for i in range(NPAIR):
                        for ps, (x, o) in zip(pso, blocks):
                            xs = slice(P * x, P * (x + 1))
                            os_ = slice(W * o, W * (o + 1))
                            nc.tensor.matmul(ps[:], ctxn[i][:, xs],
                                             wo[i][:, os_],
                                             start=(i == 0),
                                             stop=(i == NPAIR - 1))
                    for bi, (ps, (x, o)) in enumerate(zip(pso, blocks)):
                        ot = osb.tile([P, W], F32, tag="os",
                                      name=f"ot{wv}_{x}_{o}")
                        if bi % 2 == 0:
                            nc.scalar.copy(ot[:], ps[:])
                        else:
                            nc.vector.tensor_copy(ot[:], ps[:])
                        eng = nc.sync if wv == 0 else nc.scalar
                        eng.dma_start(OUT.ap()[P * x:P * (x + 1),
                                               W * o:W * (o + 1)], ot[:])

    nc.compile()
    return nc


def _get_nc():
    if "nc" not in _CACHE:
        _CACHE["nc"] = _build()
    return _CACHE["nc"]


def _esel():
    import ml_dtypes
    e = np.zeros((36, NH * 64), ml_dtypes.bfloat16)
    for h in range(NH):
        e[_zrow(h), 64 * h:64 * (h + 1)] = 1.0
    return e


def kernel(q, kv, Wq, Wkv, Wo, w=None, _trace=False):
    from concourse import bass_utils
    import ml_dtypes

    BF = ml_dtypes.bfloat16

    q = np.asarray(q, np.float32).reshape(L, DM)
    kv = np.asarray(kv, np.float32).reshape(L, DM)
    Wq = np.asarray(Wq, np.float32)
    Wkv = np.asarray(Wkv, np.float32)
    Wo = np.asarray(Wo, np.float32)

    qT = np.ascontiguousarray(q.T).astype(BF)           # [DM, L]
    kvT = np.ascontiguousarray(kv.T).astype(BF)         # [DM, L]
    WQs = np.ascontiguousarray(Wq / np.sqrt(DH)).astype(BF)
    WVK = np.concatenate([Wkv[:, DH:], Wkv[:, :DH]], axis=1)    # [Wv | Wk]
    # pack [1024,128] -> [128, 1024] tile-transposed: wvkp[:, 128f+c] = WVK[128f+p, c]
    WVKp = np.ascontiguousarray(
        WVK.reshape(NF, P, P).transpose(1, 0, 2).reshape(P, DM)).astype(BF)
    WOb = np.ascontiguousarray(Wo).astype(BF)

    in_maps = []
    for c in range(NCORES):
        kvt_c = np.zeros((DM, YW), BF)
        lo = (c - 1) * CH
        hi = (c + 2) * CH
        src_lo, src_hi = max(lo, 0), min(hi, L)
        dst_lo = src_lo - lo
        kvt_c[:, dst_lo:dst_lo + (src_hi - src_lo)] = kvT[:, src_lo:src_hi]
        in_maps.append({
            "QT": np.ascontiguousarray(qT[:, c * CH:(c + 1) * CH]),
            "KVT": kvt_c,
            "WQ": WQs,
            "WVKP": WVKp,
            "WO": WOb,
            "ESEL": _esel(),
        })

    nc = _get_nc()
    res = bass_utils.run_bass_kernel_spmd(
        nc, in_maps, core_ids=list(range(NCORES)), trace=_trace)
    if _trace:
        _CACHE["last_result"] = res

    out = np.concatenate([r["OUT"] for r in res.results], axis=0)
    return out.reshape(B, L, DM).astype(np.float32)


# revision 9
# speedup vs baseline: 1.1192x; 1.1192x over previous
"""Local (windowed) attention with shared KV head — TRN2 Bass kernel.

Problem: b=1, L=4096, d_model=1024, n_head=16, d_head=64, w=512.
  qp = (q@Wq)/8; k,v = kv@Wkv; per 512-chunk attention over {prev,self,next}
  chunks with zero-padded edges (softmax includes exp(0)=1 terms for pads);
  out = ctx @ Wo.

Sharding: sequence-parallel over the 8 chunks, one chunk per NeuronCore.
Each core recomputes the K/V projection for its 3-chunk halo (no
collectives). Edge cores receive zero-filled halo slices, which reproduces
the reference's zero-padding exactly.

The softmax exp of the 16x512x1536 score tensor per core is the ScalarE
wall (~107us on ScalarE alone), while the PE matmul stream needs ~95us.
Key structure:
  - exp split across engines: ScalarE exact exp ACTIVATE on 7 of 12
    y-tiles per head pair; DVE Schraudolph bit-trick exp on the other 5
    (one tensor_scalar computing int16(score*S + 16248.875), S=128*log2e,
    whose bit pattern IS bf16 exp(score) to ~1.8% rms; rel-err ~1.2e-2
    vs the 2e-2 gate).
  - ctx matmuls issued 2 y-tiles behind the scores matmuls so the
    in-order PE queue never puts ctx(y) (which waits on exp(y)) ahead of
    scores(y+2); otherwise every tile pays the full exp latency.
  - inputs packed host-side into [128, N] layouts and loaded as ~128KB
    DMAs (per-DMA engine rate is only ~25GB/s; ~5 in flight per queue),
    priority-ordered so the first exp fires early.
  - z rows ride the bf16 ctx staging copy; z normalize (esel broadcast
    matmul + DVE muls) runs entirely in the tail interleaved with the
    two 4-bank out-proj waves.
"""

import os
import numpy as np

B, L, DM, NH, DH, W = 1, 4096, 1024, 16, 64, 512
NCORES = 8
CH = L // NCORES        # 512 tokens per core
YW = 3 * W              # 1536 halo positions
P = 128
NF = DM // P            # 8 feature tiles
NY = YW // P            # 12 y tiles
NPAIR = NH // 2         # 8 head pairs

SCALE = float(P / np.log(2.0))          # 128*log2(e), applied in the DVE exp op
MAGIC = 16248.875                        # Schraudolph bf16 magic
DVE_Y = tuple(int(x) for x in os.environ.get("KDVE", "1,3,6,8,10").split(",")
              if x != "")                # y-tiles exp'd on DVE per pair
DELAY = 2                                # ctx matmul delay (in y-tiles)

_CACHE = {}


def _zrow(hh):
    # heads 12-15 live at partitions 32-35 so both reciprocal batches
    # start at a 32-aligned partition base (engine-op alignment rule)
    return hh if hh < 12 else 32 + (hh - 12)


def _build():
    import concourse.mybir as mybir
    import concourse.tile as tile
    from concourse import bacc
    from concourse.masks import make_identity
    from contextlib import ExitStack

    F32 = mybir.dt.float32
    BF16 = mybir.dt.bfloat16
    I16 = mybir.dt.int16
    EXP = mybir.ActivationFunctionType.Exp
    COPY = mybir.ActivationFunctionType.Copy

    nc = bacc.Bacc("TRN2", target_bir_lowering=False, debug=False)
    # all inputs packed host-side into [128, N] tile-transposed layouts
    QTP = nc.dram_tensor("QTP", [P, NF * CH], BF16, kind="ExternalInput")
    KVTP = nc.dram_tensor("KVTP", [P, 3 * NF * W], BF16, kind="ExternalInput")
    WQP = nc.dram_tensor("WQP", [P, NF * DM], BF16, kind="ExternalInput")
    WVKP = nc.dram_tensor("WVKP", [P, DM], BF16, kind="ExternalInput")
    WOP = nc.dram_tensor("WOP", [P, NF * DM], BF16, kind="ExternalInput")
    ESEL = nc.dram_tensor("ESEL", [36, NH * 64], BF16, kind="ExternalInput")
    OUT = nc.dram_tensor("OUT", [CH, DM], F32, kind="ExternalOutput")

    with tile.TileContext(nc) as tc, ExitStack() as ctx:
        perm = ctx.enter_context(tc.tile_pool(name="perm", bufs=1))

        identb = perm.tile([64, 64], F32, tag="identb")
        make_identity(nc, identb[:])
        esel = perm.tile([36, NH * 64], BF16, tag="esel")

        # --- persistent SBUF tiles
        wvkp = perm.tile([P, DM], BF16, tag="wvkp")
        wqp = perm.tile([P, NF * DM], BF16, tag="wqp")
        wop = perm.tile([P, NF * DM], BF16, tag="wop")
        qtp = perm.tile([P, NF * CH], BF16, tag="qtp")
        kvtp = perm.tile([P, 3 * NF * W], BF16, tag="kvtp")
        k3T2 = perm.tile([P, YW], BF16, tag="k3T2")
        vTs = perm.tile([64, YW], F32, tag="vTs")
        v65 = [perm.tile([P, 65], BF16, tag=f"v65_{t}", name=f"v65_{t}") for t in range(NY)]
        qpT = [perm.tile([P, CH], BF16, tag=f"qpT{m}", name=f"qpT{m}") for m in range(NF)]
        ctxn = [perm.tile([P, CH], BF16, tag=f"ctxn{i}", name=f"ctxn{i}") for i in range(NPAIR)]
        cxs = [perm.tile([P, W], BF16, tag=f"cxs{h}", name=f"cxs{h}") for h in range(NH)]
        zr16b = perm.tile([36, W], BF16, tag="zr16b")   # z rows (bf16, via cxs)
        zr32 = perm.tile([36, W], F32, tag="zr32")
        zi32 = perm.tile([36, W], F32, tag="zi32")
        zi16b = perm.tile([36, W], BF16, tag="zi16b")

        # HAM warmup: dense dummy matmuls during the DMA fill open the PE
        # clock gate (needs ~3.4us of sustained activity)
        wtile = perm.tile([P, W], BF16, tag="wtile")
        nc.gpsimd.memset(wtile[:], 1.0)
        with tc.tile_pool(name="wmps", bufs=1, space="PSUM") as wmp:
            wps = wmp.tile([P, W], F32, tag="wm")
            for _ in range(8):
                nc.tensor.matmul(wps[:], wtile[:, 0:P], wtile[:],
                                 start=True, stop=True)

        # --- input DMA fill: ~128KB pieces ([128, 512 bf16 cols]), 3
        # queues, priority-ordered (wvkp, kvt chunk0, qt, wq m0/m1 first)
        def ld(eng, tile_, dram, lo, hi):
            eng.dma_start(tile_[:, lo:hi], dram.ap()[:, lo:hi])

        KW = NF * W  # 4096 cols per kv chunk
        # sync: wvkp lo, kvt c0 p0-3, qt p0-2 | later: kvt c1/c2 p0-3
        ld(nc.sync, wvkp, WVKP, 0, W)
        for j in range(4):
            ld(nc.sync, kvtp, KVTP, W * j, W * (j + 1))
        for j in range(3):
            ld(nc.sync, qtp, QTP, W * j, W * (j + 1))
        # gpsimd: wvkp hi, kvt c0 p4-7, qt p3-5 | later: c1/c2 p4-7, esel, wop
        ld(nc.gpsimd, wvkp, WVKP, W, DM)
        for j in range(4, 8):
            ld(nc.gpsimd, kvtp, KVTP, W * j, W * (j + 1))
        for j in range(3, 6):
            ld(nc.gpsimd, qtp, QTP, W * j, W * (j + 1))
        # scalar: wq m0 (2 pieces), qt p6-7, wq m1 | later: wq m2-7
        for j in range(2):
            ld(nc.scalar, wqp, WQP, W * j, W * (j + 1))
        for j in range(6, 8):
            ld(nc.scalar, qtp, QTP, W * j, W * (j + 1))
        for j in range(2, 4):
            ld(nc.scalar, wqp, WQP, W * j, W * (j + 1))
        # background pieces for chunks 1,2 of kv
        for n in (1, 2):
            for j in range(4):
                ld(nc.sync, kvtp, KVTP, KW * n + W * j, KW * n + W * (j + 1))
            for j in range(4, 8):
                ld(nc.gpsimd, kvtp, KVTP, KW * n + W * j, KW * n + W * (j + 1))
        nc.gpsimd.dma_start(esel[:], ESEL.ap()[:, :])

        with tc.tile_pool(name="qpps", bufs=1, space="PSUM") as qpp, \
             tc.tile_pool(name="zn", bufs=4) as znp:

            def qproj(m):
                ps = qpp.tile([P, CH], F32, tag="qp")
                for f in range(NF):
                    nc.tensor.matmul(ps[:], wqp[:, (m * NF + f) * P:(m * NF + f) * P + P],
                                     qtp[:, CH * f:CH * (f + 1)],
                                     start=(f == 0), stop=(f == NF - 1))
                with nc.allow_low_precision(reason="bf16 attention pipeline"):
                    nc.vector.tensor_copy(qpT[m][:], ps[:])

            with tc.tile_pool(name="ph0ps", bufs=2, space="PSUM") as ph0, \
                 tc.tile_pool(name="tpps", bufs=2, space="PSUM") as tpp:

                def kvproj(n):
                    ps = ph0.tile([P, W], F32, tag="kvp")
                    for f in range(NF):
                        nc.tensor.matmul(ps[:], wvkp[:, P * f:P * (f + 1)],
                                         kvtp[:, (n * NF + f) * W:(n * NF + f) * W + W],
                                         start=(f == 0), stop=(f == NF - 1))
                    ns = slice(W * n, W * (n + 1))
                    with nc.allow_low_precision(reason="bf16 attention pipeline"):
                        nc.vector.tensor_copy(vTs[:, ns], ps[0:64, :])
                        nc.vector.tensor_copy(k3T2[64:128, ns], ps[64:128, :])
                    # duplicate kT into the low partition half (partition remap)
                    nc.scalar.dma_start(k3T2[0:64, ns], k3T2[64:128, ns])
                    # v65 tiles for this chunk: PE transpose of vT + ones col
                    for t in range(4 * n, 4 * (n + 1)):
                        tp = tpp.tile([P, 64], F32, tag="tp")
                        nc.tensor.transpose(tp[:], vTs[:, P * t:P * (t + 1)],
                                            identb[:])
                        with nc.allow_low_precision(reason="bf16 attention pipeline"):
                            nc.vector.tensor_copy(v65[t][:, 0:64], tp[:])
                        nc.vector.memset(v65[t][:, 64:65], 1.0)

                # interleave kv-proj w-chunks with the first q-proj tiles
                kvproj(0)
                qproj(0)
                kvproj(1)
                qproj(1)
                kvproj(2)

            # background: wq m2-7 on scalar, wo on gpsimd (needed from pair 6)
            for j in range(4, 16):
                ld(nc.scalar, wqp, WQP, W * j, W * (j + 1))
            for j in range(NF):
                ld(nc.gpsimd, wop, WOP, DM * j, DM * (j + 1))

            def z_recip(lo, hi):
                # zr16b rows -> fp32 -> 1/x -> bf16 (for the esel broadcast)
                nc.vector.tensor_copy(zr32[lo:hi, :], zr16b[lo:hi, :])
                nc.vector.reciprocal(zi32[lo:hi, :], zr32[lo:hi, :])
                with nc.allow_low_precision(reason="softmax denom"):
                    nc.vector.tensor_copy(zi16b[lo:hi, :], zi32[lo:hi, :])

            def z_apply(heads):
                for hh in heads:
                    i, h = hh // 2, hh % 2
                    lo = 0 if hh < 12 else 32
                    hi = 12 if hh < 12 else 36
                    zb = qpp.tile([P, W], F32, tag="qp")
                    nc.tensor.matmul(zb[0:64, :],
                                     esel[lo:hi, 64 * hh:64 * (hh + 1)],
                                     zi16b[lo:hi, :], start=True, stop=True)
                    if h == 0:
                        with nc.allow_low_precision(reason="bf16 ctx"):
                            nc.vector.tensor_mul(ctxn[i][0:64, :],
                                                 cxs[hh][0:64, :], zb[0:64, :])
                    else:
                        cbt = znp.tile([64, W], BF16, tag="cbt")
                        with nc.allow_low_precision(reason="bf16 ctx"):
                            nc.vector.tensor_mul(cbt[:], cxs[hh][0:64, :],
                                                 zb[0:64, :])
                        nc.sync.dma_start(ctxn[i][64:128, :], cbt[:])

            # --- attention per head pair; scores for the two heads
            # interleave into one psum tile; exp split ScalarE/DVE; ctx
            # matmuls trail the scores stream by DELAY y-tiles
            attn = ExitStack()
            scp = attn.enter_context(tc.tile_pool(name="scps", bufs=2, space="PSUM"))
            cxp = attn.enter_context(tc.tile_pool(name="cxps", bufs=3, space="PSUM"))
            ptp = attn.enter_context(tc.tile_pool(name="pt", bufs=4))
            for i in range(NPAIR):
                cxA = cxp.tile([P, W], F32, tag="cx")
                cxB = cxp.tile([P, W], F32, tag="cx")
                pabs = [None] * NY

                def ctx_mm(y):
                    pa = pabs[y]
                    st = (y == 0)
                    sp = (y == NY - 1)
                    nc.tensor.matmul(cxA[0:65, :], v65[y][:], pa[:, 0:W],
                                     start=st, stop=sp)
                    nc.tensor.matmul(cxB[0:65, :], v65[y][:], pa[:, W:2 * W],
                                     start=st, stop=sp)

                for y in range(NY):
                    ys = slice(P * y, P * (y + 1))
                    sc = scp.tile([P, 2 * W], F32, tag="sc")
                    nc.tensor.matmul(sc[:, 0:W], k3T2[0:64, ys],
                                     qpT[i][0:64, :], start=True, stop=True,
                                     tile_position=(0, 0))
                    nc.tensor.matmul(sc[:, W:2 * W], k3T2[64:128, ys],
                                     qpT[i][64:128, :], start=True, stop=True,
                                     tile_position=(64, 0))
                    pab = ptp.tile([P, 2 * W], BF16, tag="pt")
                    pabs[y] = pab
                    with nc.allow_low_precision(reason="bf16 probs"):
                        if y in DVE_Y:
                            nc.vector.tensor_scalar(
                                pab[:].bitcast(I16), sc[:], SCALE, MAGIC,
                                op0=mybir.AluOpType.mult,
                                op1=mybir.AluOpType.add)
                        else:
                            nc.scalar.activation(pab[:], sc[:], EXP)
                    if y >= DELAY:
                        ctx_mm(y - DELAY)
                for y in range(NY - DELAY, NY):
                    ctx_mm(y)

                # stage unnormalized ctx + Z row out of PSUM (bf16, one copy
                # per head; row 64 is the Z row from the v65 ones column)
                for h, cx in ((0, cxA), (1, cxB)):
                    hh = 2 * i + h
                    with nc.allow_low_precision(reason="bf16 ctx"):
                        if h == 0:
                            nc.scalar.activation(cxs[hh][0:65, :], cx[0:65, :],
                                                 COPY)
                        else:
                            nc.vector.tensor_copy(cxs[hh][0:65, :], cx[0:65, :])
                    nc.sync.dma_start(zr16b[_zrow(hh):_zrow(hh) + 1, :],
                                      cxs[hh][64:65, :])
                if i + 2 < NF:
                    qproj(i + 2)
                if i == 5:
                    z_recip(0, 12)          # heads 0-11, overlaps pairs 6-7
            attn.close()

            # --- tail: z normalize + output projection (two 4-bank waves)
            z_apply(list(range(12)))
            z_recip(32, 36)                  # heads 12-15
            z_apply([12, 13, 14, 15])

            with tc.tile_pool(name="opps", bufs=4, space="PSUM") as opp, \
                 tc.tile_pool(name="osb", bufs=8) as osb:
                allblk = [(x, o) for x in range(4) for o in range(2)]
                for wv in range(2):
                    blocks = allblk[4 * wv:4 * (wv + 1)]
                    pso = [opp.tile([P, W], F32, tag="op", name=f"op{wv}_{b}")
                           for b in range(4)]
                    for i in range(NPAIR):
                        for ps, (x, o) in zip(pso, blocks):
                            xs = slice(P * x, P * (x + 1))
                            os_ = slice(DM * i + W * o, DM * i + W * (o + 1))
                            nc.tensor.matmul(ps[:], ctxn[i][:, xs],
                                             wop[:, os_],
                                             start=(i == 0),
                                             stop=(i == NPAIR - 1))
                    for bi, (ps, (x, o)) in enumerate(zip(pso, blocks)):
                        ot = osb.tile([P, W], F32, tag="os",
                                      name=f"ot{wv}_{x}_{o}")
                        if bi % 2 == 0:
                            nc.scalar.copy(ot[:], ps[:])
                        else:
                            nc.vector.tensor_copy(ot[:], ps[:])
                        eng = nc.sync if wv == 0 else nc.scalar
                        eng.dma_start(OUT.ap()[P * x:P * (x + 1),
                                               W * o:W * (o + 1)], ot[:])

    nc.compile()
    return nc


def _get_nc():
    if "nc" not in _CACHE:
        _CACHE["nc"] = _build()
    return _CACHE["nc"]


def _esel():
    import ml_dtypes
    e = np.zeros((36, NH * 64), ml_dtypes.bfloat16)
    for h in range(NH):
        e[_zrow(h), 64 * h:64 * (h + 1)] = 1.0
    return e


def _prep_host(q, kv, Wq, Wkv, Wo):
    """Pack all inputs into the [128, N] tile-transposed dram layouts."""
    import ml_dtypes
    BF = ml_dtypes.bfloat16

    q = np.asarray(q, np.float32).reshape(L, DM)
    kv = np.asarray(kv, np.float32).reshape(L, DM)
    Wq = np.asarray(Wq, np.float32)
    Wkv = np.asarray(Wkv, np.float32)
    Wo = np.asarray(Wo, np.float32)

    qT = np.ascontiguousarray(q.T).astype(BF)           # [DM, L]
    kvT = np.ascontiguousarray(kv.T).astype(BF)         # [DM, L]
    WQs = (Wq / np.sqrt(DH)).astype(BF)
    # WQP[p, (m*8+f)*128 + c] = WQs[128f+p, 128m+c]
    WQP = np.ascontiguousarray(
        WQs.reshape(NF, P, NF, P).transpose(1, 2, 0, 3).reshape(P, NF * DM))
    WVK = np.concatenate([Wkv[:, DH:], Wkv[:, :DH]], axis=1).astype(BF)  # [Wv|Wk]
    WVKP = np.ascontiguousarray(
        WVK.reshape(NF, P, P).transpose(1, 0, 2).reshape(P, DM))
    # WOP[p, 1024*i + c] = Wo[128i+p, c]
    WOP = np.ascontiguousarray(
        Wo.astype(BF).reshape(NF, P, DM).transpose(1, 0, 2).reshape(P, NF * DM))

    in_maps = []
    for c in range(NCORES):
        kvt_c = np.zeros((DM, YW), BF)
        lo = (c - 1) * CH
        hi = (c + 2) * CH
        src_lo, src_hi = max(lo, 0), min(hi, L)
        dst_lo = src_lo - lo
        kvt_c[:, dst_lo:dst_lo + (src_hi - src_lo)] = kvT[:, src_lo:src_hi]
        # KVTP[p, (n*8+f)*512 + c] = kvt_c[128f+p, 512n+c]
        KVTP = np.ascontiguousarray(
            kvt_c.reshape(NF, P, 3, W).transpose(1, 2, 0, 3).reshape(P, 3 * NF * W))
        qt_c = qT[:, c * CH:(c + 1) * CH]
        QTP = np.ascontiguousarray(
            qt_c.reshape(NF, P, CH).transpose(1, 0, 2).reshape(P, NF * CH))
        in_maps.append({
            "QTP": QTP,
            "KVTP": KVTP,
            "WQP": WQP,
            "WVKP": WVKP,
            "WOP": WOP,
            "ESEL": _esel(),
        })
    return in_maps


def kernel(q, kv, Wq, Wkv, Wo, w=None, _trace=False):
    from concourse import bass_utils

    in_maps = _prep_host(q, kv, Wq, Wkv, Wo)
    nc = _get_nc()
    res = bass_utils.run_bass_kernel_spmd(
        nc, in_maps, core_ids=list(range(NCORES)), trace=_trace)
    if _trace:
        _CACHE["last_result"] = res

    out = np.concatenate([r["OUT"] for r in res.results], axis=0)
    return out.reshape(B, L, DM).astype(np.float32)


# revision 10
# speedup vs baseline: 1.2115x; 1.0824x over previous
"""Local (windowed) attention with shared KV head — TRN2 Bass kernel.

Problem: b=1, L=4096, d_model=1024, n_head=16, d_head=64, w=512.
  qp = (q@Wq)/8; k,v = kv@Wkv; per 512-chunk attention over {prev,self,next}
  chunks with zero-padded edges (softmax includes exp(0)=1 terms for pads);
  out = ctx @ Wo.

Sharding: sequence-parallel over the 8 chunks, one chunk per NeuronCore.
Each core recomputes the K/V projection for its 3-chunk halo (no
collectives). Edge cores receive zero-filled halo slices, which reproduces
the reference's zero-padding exactly.

The softmax exp of the 16x512x1536 score tensor per core is the ScalarE
wall (~107us on ScalarE alone), while the PE matmul stream needs ~95us.
Key structure:
  - exp split across engines: ScalarE exact exp ACTIVATE on 7 of 12
    y-tiles per head pair; DVE Schraudolph bit-trick exp on the other 5
    (one tensor_scalar computing int16(score*S + 16248.875), S=128*log2e,
    whose bit pattern IS bf16 exp(score) to ~1.8% rms; rel-err ~1.2e-2
    vs the 2e-2 gate).
  - ctx matmuls issued 2 y-tiles behind the scores matmuls so the
    in-order PE queue never puts ctx(y) (which waits on exp(y)) ahead of
    scores(y+2); otherwise every tile pays the full exp latency.
  - inputs packed host-side into [128, N] layouts and loaded as ~128KB
    DMAs (per-DMA engine rate is only ~25GB/s; ~5 in flight per queue),
    priority-ordered so the first exp fires early.
  - z rows ride the bf16 ctx staging copy; z normalize (esel broadcast
    matmul + DVE muls) runs entirely in the tail interleaved with the
    two 4-bank out-proj waves.
"""

import os
import numpy as np

B, L, DM, NH, DH, W = 1, 4096, 1024, 16, 64, 512
NCORES = 8
CH = L // NCORES        # 512 tokens per core
YW = 3 * W              # 1536 halo positions
P = 128
NF = DM // P            # 8 feature tiles
NY = YW // P            # 12 y tiles
NPAIR = NH // 2         # 8 head pairs

SCALE = float(P / np.log(2.0))          # 128*log2(e), applied in the DVE exp op
MAGIC = 16248.875                        # Schraudolph bf16 magic
DVE_Y = tuple(int(x) for x in os.environ.get("KDVE", "1,3,6,8,10").split(",")
              if x != "")                # y-tiles exp'd on DVE per pair
DELAY = 2                                # ctx matmul delay (in y-tiles)

_CACHE = {}


def _zrow(hh):
    # heads 12-15 live at partitions 32-35 so both reciprocal batches
    # start at a 32-aligned partition base (engine-op alignment rule)
    return hh if hh < 12 else 32 + (hh - 12)


def _build():
    import concourse.mybir as mybir
    import concourse.tile as tile
    from concourse import bacc
    from concourse.masks import make_identity
    from contextlib import ExitStack

    F32 = mybir.dt.float32
    BF16 = mybir.dt.bfloat16
    I16 = mybir.dt.int16
    EXP = mybir.ActivationFunctionType.Exp
    COPY = mybir.ActivationFunctionType.Copy

    nc = bacc.Bacc("TRN2", target_bir_lowering=False, debug=False)
    # all inputs packed host-side into [128, N] tile-transposed layouts
    QTP = nc.dram_tensor("QTP", [P, NF * CH], BF16, kind="ExternalInput")
    KVTP = nc.dram_tensor("KVTP", [P, 3 * NF * W], BF16, kind="ExternalInput")
    WQP = nc.dram_tensor("WQP", [P, NF * DM], BF16, kind="ExternalInput")
    WVKP = nc.dram_tensor("WVKP", [P, DM], BF16, kind="ExternalInput")
    WOP = nc.dram_tensor("WOP", [P, NF * DM], BF16, kind="ExternalInput")
    ESEL = nc.dram_tensor("ESEL", [36, NH * 64], BF16, kind="ExternalInput")
    OUT = nc.dram_tensor("OUT", [CH, DM], F32, kind="ExternalOutput")

    with tile.TileContext(nc) as tc, ExitStack() as ctx:
        perm = ctx.enter_context(tc.tile_pool(name="perm", bufs=1))

        identb = perm.tile([64, 64], F32, tag="identb")
        make_identity(nc, identb[:])
        esel = perm.tile([36, NH * 64], BF16, tag="esel")

        # --- persistent SBUF tiles
        wvkp = perm.tile([P, DM], BF16, tag="wvkp")
        wqp = perm.tile([P, NF * DM], BF16, tag="wqp")
        wop = perm.tile([P, NF * DM], BF16, tag="wop")
        qtp = perm.tile([P, NF * CH], BF16, tag="qtp")
        kvtp = perm.tile([P, 3 * NF * W], BF16, tag="kvtp")
        k3T2 = perm.tile([P, YW], BF16, tag="k3T2")
        vTs = perm.tile([64, YW], F32, tag="vTs")
        v65 = [perm.tile([P, 65], BF16, tag=f"v65_{t}", name=f"v65_{t}") for t in range(NY)]
        qpT = [perm.tile([P, CH], BF16, tag=f"qpT{m}", name=f"qpT{m}") for m in range(NF)]
        ctxn = [perm.tile([P, CH], BF16, tag=f"ctxn{i}", name=f"ctxn{i}") for i in range(NPAIR)]
        cxs = [perm.tile([P, W], BF16, tag=f"cxs{h}", name=f"cxs{h}") for h in range(NH)]
        zr16b = perm.tile([36, W], BF16, tag="zr16b")   # z rows (bf16, via cxs)
        zr32 = perm.tile([36, W], F32, tag="zr32")
        zi32 = perm.tile([36, W], F32, tag="zi32")
        zi16b = perm.tile([36, W], BF16, tag="zi16b")

        # HAM warmup: dense dummy matmuls during the DMA fill open the PE
        # clock gate (needs ~3.4us of sustained activity)
        wtile = perm.tile([P, W], BF16, tag="wtile")
        nc.gpsimd.memset(wtile[:], 1.0)
        with tc.tile_pool(name="wmps", bufs=1, space="PSUM") as wmp:
            wps = wmp.tile([P, W], F32, tag="wm")
            for _ in range(8):
                nc.tensor.matmul(wps[:], wtile[:, 0:P], wtile[:],
                                 start=True, stop=True)

        # --- input DMA fill: ~128KB pieces ([128, 512 bf16 cols]), 3
        # queues, priority-ordered (wvkp, kvt chunk0, qt, wq m0/m1 first)
        def ld(eng, tile_, dram, lo, hi):
            eng.dma_start(tile_[:, lo:hi], dram.ap()[:, lo:hi])

        KW = NF * W  # 4096 cols per kv chunk
        # sync: wvkp lo, kvt c0 p0-3, qt p0-2 | later: kvt c1/c2 p0-3
        ld(nc.sync, wvkp, WVKP, 0, W)
        for j in range(4):
            ld(nc.sync, kvtp, KVTP, W * j, W * (j + 1))
        for j in range(3):
            ld(nc.sync, qtp, QTP, W * j, W * (j + 1))
        # gpsimd: wvkp hi, kvt c0 p4-7, qt p3-5 | later: c1/c2 p4-7, esel, wop
        ld(nc.gpsimd, wvkp, WVKP, W, DM)
        for j in range(4, 8):
            ld(nc.gpsimd, kvtp, KVTP, W * j, W * (j + 1))
        for j in range(3, 6):
            ld(nc.gpsimd, qtp, QTP, W * j, W * (j + 1))
        # scalar: wq m0 (2 pieces), qt p6-7, wq m1 | later: wq m2-7
        for j in range(2):
            ld(nc.scalar, wqp, WQP, W * j, W * (j + 1))
        for j in range(6, 8):
            ld(nc.scalar, qtp, QTP, W * j, W * (j + 1))
        for j in range(2, 4):
            ld(nc.scalar, wqp, WQP, W * j, W * (j + 1))
        # background pieces for chunks 1,2 of kv
        for n in (1, 2):
            for j in range(4):
                ld(nc.sync, kvtp, KVTP, KW * n + W * j, KW * n + W * (j + 1))
            for j in range(4, 8):
                ld(nc.gpsimd, kvtp, KVTP, KW * n + W * j, KW * n + W * (j + 1))
        nc.gpsimd.dma_start(esel[:], ESEL.ap()[:, :])

        with tc.tile_pool(name="zn", bufs=4) as znp:

            with tc.tile_pool(name="ph0ps", bufs=2, space="PSUM") as ph0, \
                 tc.tile_pool(name="tpps", bufs=2, space="PSUM") as tpp, \
                 tc.tile_pool(name="eqps", bufs=1, space="PSUM") as eqp:

                def qproj0(m):
                    ps = eqp.tile([P, CH], F32, tag="eq")
                    for f in range(NF):
                        nc.tensor.matmul(ps[:], wqp[:, (m * NF + f) * P:(m * NF + f) * P + P],
                                         qtp[:, CH * f:CH * (f + 1)],
                                         start=(f == 0), stop=(f == NF - 1))
                    with nc.allow_low_precision(reason="bf16 attention pipeline"):
                        nc.vector.tensor_copy(qpT[m][:], ps[:])

                def kvproj(n):
                    ps = ph0.tile([P, W], F32, tag="kvp")
                    for f in range(NF):
                        nc.tensor.matmul(ps[:], wvkp[:, P * f:P * (f + 1)],
                                         kvtp[:, (n * NF + f) * W:(n * NF + f) * W + W],
                                         start=(f == 0), stop=(f == NF - 1))
                    ns = slice(W * n, W * (n + 1))
                    with nc.allow_low_precision(reason="bf16 attention pipeline"):
                        nc.vector.tensor_copy(vTs[:, ns], ps[0:64, :])
                        nc.vector.tensor_copy(k3T2[64:128, ns], ps[64:128, :])
                    # duplicate kT into the low partition half (partition remap)
                    nc.scalar.dma_start(k3T2[0:64, ns], k3T2[64:128, ns])

                def v65build(n):
                    # v65 tiles for chunk n: PE transpose of vT + ones col
                    for t in range(4 * n, 4 * (n + 1)):
                        tp = tpp.tile([P, 64], F32, tag="tp")
                        nc.tensor.transpose(tp[:], vTs[:, P * t:P * (t + 1)],
                                            identb[:])
                        with nc.allow_low_precision(reason="bf16 attention pipeline"):
                            nc.vector.tensor_copy(v65[t][:, 0:64], tp[:])
                        nc.vector.memset(v65[t][:, 64:65], 1.0)

                # kv-proj chunks interleaved with the first q-proj tiles;
                # v65 transposes follow qproj so they don't block it on the
                # in-order PE queue
                kvproj(0)
                qproj0(0)
                v65build(0)
                kvproj(1)
                qproj0(1)
                v65build(1)
                kvproj(2)
                v65build(2)

            # background: wq m2-7 and wo on gpsimd (scalar queue must stay
            # clean during attention -- DMA issues stall the ScalarE queue)
            for j in range(4, 16):
                ld(nc.gpsimd, wqp, WQP, W * j, W * (j + 1))
            for j in range(NF):
                ld(nc.gpsimd, wop, WOP, DM * j, DM * (j + 1))

            def z_recip(lo, hi):
                # zr16b rows -> fp32 -> 1/x -> bf16 (for the esel broadcast)
                nc.vector.tensor_copy(zr32[lo:hi, :], zr16b[lo:hi, :])
                nc.vector.reciprocal(zi32[lo:hi, :], zr32[lo:hi, :])
                with nc.allow_low_precision(reason="softmax denom"):
                    nc.vector.tensor_copy(zi16b[lo:hi, :], zi32[lo:hi, :])

            def z_apply(heads, zbp):
                for hh in heads:
                    i, h = hh // 2, hh % 2
                    lo = 0 if hh < 12 else 32
                    hi = 12 if hh < 12 else 36
                    zb = zbp.tile([P, W], F32, tag="zb")
                    nc.tensor.matmul(zb[0:64, :],
                                     esel[lo:hi, 64 * hh:64 * (hh + 1)],
                                     zi16b[lo:hi, :], start=True, stop=True)
                    if h == 0:
                        with nc.allow_low_precision(reason="bf16 ctx"):
                            nc.vector.tensor_mul(ctxn[i][0:64, :],
                                                 cxs[hh][0:64, :], zb[0:64, :])
                    else:
                        cbt = znp.tile([64, W], BF16, tag="cbt")
                        with nc.allow_low_precision(reason="bf16 ctx"):
                            nc.vector.tensor_mul(cbt[:], cxs[hh][0:64, :],
                                                 zb[0:64, :])
                        nc.sync.dma_start(ctxn[i][64:128, :], cbt[:])

            # --- attention per head pair; scores for the two heads
            # interleave into one psum tile; exp split ScalarE/DVE; ctx
            # matmuls trail the scores stream by DELAY y-tiles
            attn = ExitStack()
            scp = attn.enter_context(tc.tile_pool(name="scps", bufs=3, space="PSUM"))
            cxp = attn.enter_context(tc.tile_pool(name="cxps", bufs=2, space="PSUM"))
            ptp = attn.enter_context(tc.tile_pool(name="pt", bufs=4))

            def qproj(m):
                ps = scp.tile([P, 2 * W], F32, tag="sc")
                for f in range(NF):
                    nc.tensor.matmul(ps[:, 0:CH], wqp[:, (m * NF + f) * P:(m * NF + f) * P + P],
                                     qtp[:, CH * f:CH * (f + 1)],
                                     start=(f == 0), stop=(f == NF - 1))
                with nc.allow_low_precision(reason="bf16 attention pipeline"):
                    nc.vector.tensor_copy(qpT[m][:], ps[:, 0:CH])
            for i in range(NPAIR):
                cxA = cxp.tile([P, W], F32, tag="cx")
                cxB = cxp.tile([P, W], F32, tag="cx")
                pabs = [None] * NY

                def ctx_mm(y):
                    pa = pabs[y]
                    st = (y == 0)
                    sp = (y == NY - 1)
                    nc.tensor.matmul(cxA[0:65, :], v65[y][:], pa[:, 0:W],
                                     start=st, stop=sp)
                    nc.tensor.matmul(cxB[0:65, :], v65[y][:], pa[:, W:2 * W],
                                     start=st, stop=sp)

                for y in range(NY):
                    ys = slice(P * y, P * (y + 1))
                    sc = scp.tile([P, 2 * W], F32, tag="sc")
                    nc.tensor.matmul(sc[:, 0:W], k3T2[0:64, ys],
                                     qpT[i][0:64, :], start=True, stop=True,
                                     tile_position=(0, 0))
                    nc.tensor.matmul(sc[:, W:2 * W], k3T2[64:128, ys],
                                     qpT[i][64:128, :], start=True, stop=True,
                                     tile_position=(64, 0))
                    pab = ptp.tile([P, 2 * W], BF16, tag="pt")
                    pabs[y] = pab
                    with nc.allow_low_precision(reason="bf16 probs"):
                        if y in DVE_Y:
                            nc.vector.tensor_scalar(
                                pab[:].bitcast(I16), sc[:], SCALE, MAGIC,
                                op0=mybir.AluOpType.mult,
                                op1=mybir.AluOpType.add)
                        else:
                            nc.scalar.activation(pab[:], sc[:], EXP)
                    if y >= DELAY:
                        ctx_mm(y - DELAY)
                for y in range(NY - DELAY, NY):
                    ctx_mm(y)

                # stage unnormalized ctx + Z row out of PSUM (bf16, one copy
                # per head; row 64 is the Z row from the v65 ones column)
                for h, cx in ((0, cxA), (1, cxB)):
                    hh = 2 * i + h
                    with nc.allow_low_precision(reason="bf16 ctx"):
                        if h == 0:
                            nc.scalar.activation(cxs[hh][0:65, :], cx[0:65, :],
                                                 COPY)
                        else:
                            nc.vector.tensor_copy(cxs[hh][0:65, :], cx[0:65, :])
                    nc.sync.dma_start(zr16b[_zrow(hh):_zrow(hh) + 1, :],
                                      cxs[hh][64:65, :])
                if i + 2 < NF:
                    qproj(i + 2)
                if i == 5:
                    z_recip(0, 12)          # heads 0-11, overlaps pairs 6-7
            attn.close()

            # --- tail: z normalize + output projection (two 4-bank waves)
            with tc.tile_pool(name="zbps", bufs=4, space="PSUM") as zbp, \
                 tc.tile_pool(name="opps", bufs=4, space="PSUM") as opp, \
                 tc.tile_pool(name="osb", bufs=8) as osb:
                z_apply(list(range(12)), zbp)
                z_recip(32, 36)                  # heads 12-15
                z_apply([12, 13, 14, 15], zbp)
                allblk = [(x, o) for x in range(4) for o in range(2)]
                for wv in range(2):
                    blocks = allblk[4 * wv:4 * (wv + 1)]
                    pso = [opp.tile([P, W], F32, tag="op", name=f"op{wv}_{b}")
                           for b in range(4)]
                    for i in range(NPAIR):
                        for ps, (x, o) in zip(pso, blocks):
                            xs = slice(P * x, P * (x + 1))
                            os_ = slice(DM * i + W * o, DM * i + W * (o + 1))
                            nc.tensor.matmul(ps[:], ctxn[i][:, xs],
                                             wop[:, os_],
                                             start=(i == 0),
                                             stop=(i == NPAIR - 1))
                    for bi, (ps, (x, o)) in enumerate(zip(pso, blocks)):
                        ot = osb.tile([P, W], F32, tag="os",
                                      name=f"ot{wv}_{x}_{o}")
                        if bi % 2 == 0:
                            nc.scalar.copy(ot[:], ps[:])
                        else:
                            nc.vector.tensor_copy(ot[:], ps[:])
                        eng = nc.sync if wv == 0 else nc.scalar
                        eng.dma_start(OUT.ap()[P * x:P * (x + 1),
                                               W * o:W * (o + 1)], ot[:])

    nc.compile()
    return nc


def _get_nc():
    if "nc" not in _CACHE:
        _CACHE["nc"] = _build()
    return _CACHE["nc"]


def _esel():
    import ml_dtypes
    e = np.zeros((36, NH * 64), ml_dtypes.bfloat16)
    for h in range(NH):
        e[_zrow(h), 64 * h:64 * (h + 1)] = 1.0
    return e


def _prep_host(q, kv, Wq, Wkv, Wo):
    """Pack all inputs into the [128, N] tile-transposed dram layouts."""
    import ml_dtypes
    BF = ml_dtypes.bfloat16

    q = np.asarray(q, np.float32).reshape(L, DM)
    kv = np.asarray(kv, np.float32).reshape(L, DM)
    Wq = np.asarray(Wq, np.float32)
    Wkv = np.asarray(Wkv, np.float32)
    Wo = np.asarray(Wo, np.float32)

    qT = np.ascontiguousarray(q.T).astype(BF)           # [DM, L]
    kvT = np.ascontiguousarray(kv.T).astype(BF)         # [DM, L]
    WQs = (Wq / np.sqrt(DH)).astype(BF)
    # WQP[p, (m*8+f)*128 + c] = WQs[128f+p, 128m+c]
    WQP = np.ascontiguousarray(
        WQs.reshape(NF, P, NF, P).transpose(1, 2, 0, 3).reshape(P, NF * DM))
    WVK = np.concatenate([Wkv[:, DH:], Wkv[:, :DH]], axis=1).astype(BF)  # [Wv|Wk]
    WVKP = np.ascontiguousarray(
        WVK.reshape(NF, P, P).transpose(1, 0, 2).reshape(P, DM))
    # WOP[p, 1024*i + c] = Wo[128i+p, c]
    WOP = np.ascontiguousarray(
        Wo.astype(BF).reshape(NF, P, DM).transpose(1, 0, 2).reshape(P, NF * DM))

    in_maps = []
    for c in range(NCORES):
        kvt_c = np.zeros((DM, YW), BF)
        lo = (c - 1) * CH
        hi = (c + 2) * CH
        src_lo, src_hi = max(lo, 0), min(hi, L)
        dst_lo = src_lo - lo
        kvt_c[:, dst_lo:dst_lo + (src_hi - src_lo)] = kvT[:, src_lo:src_hi]
        # KVTP[p, (n*8+f)*512 + c] = kvt_c[128f+p, 512n+c]
        KVTP = np.ascontiguousarray(
            kvt_c.reshape(NF, P, 3, W).transpose(1, 2, 0, 3).reshape(P, 3 * NF * W))
        qt_c = qT[:, c * CH:(c + 1) * CH]
        QTP = np.ascontiguousarray(
            qt_c.reshape(NF, P, CH).transpose(1, 0, 2).reshape(P, NF * CH))
        in_maps.append({
            "QTP": QTP,
            "KVTP": KVTP,
            "WQP": WQP,
            "WVKP": WVKP,
            "WOP": WOP,
            "ESEL": _esel(),
        })
    return in_maps


def kernel(q, kv, Wq, Wkv, Wo, w=None, _trace=False):
    from concourse import bass_utils

    in_maps = _prep_host(q, kv, Wq, Wkv, Wo)
    nc = _get_nc()
    res = bass_utils.run_bass_kernel_spmd(
        nc, in_maps, core_ids=list(range(NCORES)), trace=_trace)
    if _trace:
        _CACHE["last_result"] = res

    out = np.concatenate([r["OUT"] for r in res.results], axis=0)
    return out.reshape(B, L, DM).astype(np.float32)
